# revision 1
# baseline (speedup 1.0000x reference)
"""Trainium2 Bass kernel for nn_DropLearner (gnn_message_passing).

aug_edge_weight = sigmoid((logit(eps) + MLP([head|tail|rel])) / T)

Strategy (8 NeuronCores, data-parallel over edges):
  - Edges sharded 62500/core, padded to 63488 = 31 groups x 2048.
  - all_embed gathered fp32 (512B rows) per-edge via indirect DMA
    (int32 row indices); HW consumes one index per output partition, so
    each instruction gathers 128 rows (992 instructions/core — the
    Pool-engine fixed cost per instruction is the kernel's floor).
  - Gathered edge-major tiles are transposed feature-major on the
    TensorEngine (fp32 128x128 blocks into PSUM, copied out via DVE/ACT).
  - MLP: h.T[192, 512] accumulated in PSUM from 3 matmuls per 96-half:
    W1h.T @ headT + W1t.T @ tailT + Rb.T @ onehot(type); relu-copied to
    SBUF; weight = W2 @ h via matmuls into packed PSUM rows
    (tile_position col-packing).
  - Per-edge weights staged to DRAM, re-read as [128, 496] for bulk
    gating (Ln/sigmoid on the scalar engine).
Precision: full fp32 (PE slack under the gather-instruction floor) -> ~1e-6 max
relative error vs the fp32 reference.
"""
import sys
sys.path.insert(0, "/opt/trn_rl_repo")

import contextlib
import numpy as np

import concourse.bacc as bacc
import concourse.bass as bass
import concourse.mybir as mybir
import concourse.tile as tile
from concourse.bass_utils import run_bass_kernel_spmd

# ---- problem constants (hardcoded per task contract) ----
N_NODES = 100000
D = 128           # node dim
N_REL = 32
E = 500000
H = 192           # 3 * mlp_dim
TEMP = 0.5
BIAS = 1e-4

NCORES = 8
EC = E // NCORES          # 62500 edges per core
GROUP = 2048              # edges per gather group
NG = 31                   # groups per core
EP = NG * GROUP           # 63488 padded edges per core
KIDX = GROUP // 128       # 16 idx columns per group
F = EP // 128             # 496 columns in the [128, F] final layout
NCHUNK = NG * 4           # 124 chunks of 512

F16 = mybir.dt.float16
BF16 = mybir.dt.bfloat16
F32 = mybir.dt.float32
I32 = mybir.dt.int32

_CACHE = {}


def _build_program():
    nc = bacc.Bacc("TRN2", target_bir_lowering=False, debug=False,
                   num_devices=NCORES)
    tab = nc.dram_tensor("tab", [N_NODES, D], F32, kind="ExternalInput").ap()
    idxh = nc.dram_tensor("idxh", [128, F], I32, kind="ExternalInput").ap()
    idxt = nc.dram_tensor("idxt", [128, F], I32, kind="ExternalInput").ap()
    onehot = nc.dram_tensor("onehot", [NG, N_REL, GROUP], BF16, kind="ExternalInput").ap()
    u_in = nc.dram_tensor("u", [EP], F32, kind="ExternalInput").ap()
    w1ht = nc.dram_tensor("w1ht", [D, H], F32, kind="ExternalInput").ap()
    w1tt = nc.dram_tensor("w1tt", [D, H], F32, kind="ExternalInput").ap()
    rbt = nc.dram_tensor("rbt", [N_REL, 2 * H], BF16, kind="ExternalInput").ap()  # [hi | lo]
    w2c = nc.dram_tensor("w2c", [96, 2], F32, kind="ExternalInput").ap()
    b2b = nc.dram_tensor("b2b", [128, 1], F32, kind="ExternalInput").ap()
    gate = nc.dram_tensor("gate", [EP], F32, kind="ExternalOutput").ap()

    RELU = mybir.ActivationFunctionType.Relu
    LN = mybir.ActivationFunctionType.Ln
    SIG = mybir.ActivationFunctionType.Sigmoid

    with tile.TileContext(nc) as tc, contextlib.ExitStack() as ctx:
        constp = ctx.enter_context(tc.tile_pool(name="const", bufs=1))
        gathp = ctx.enter_context(tc.tile_pool(name="gath", bufs=2))
        onep = ctx.enter_context(tc.tile_pool(name="onep", bufs=2))
        xtp = ctx.enter_context(tc.tile_pool(name="xt", bufs=3))
        hps = ctx.enter_context(tc.tile_pool(name="hps", bufs=2, space="PSUM"))
        wps = ctx.enter_context(tc.tile_pool(name="wps", bufs=2, space="PSUM"))
        xpp = ctx.enter_context(tc.tile_pool(name="xpp", bufs=2, space="PSUM"))
        hsbp = ctx.enter_context(tc.tile_pool(name="hsb", bufs=3))
        wsbp = ctx.enter_context(tc.tile_pool(name="wsb", bufs=2))
        finp = ctx.enter_context(tc.tile_pool(name="fin", bufs=1))
        dramp = ctx.enter_context(tc.tile_pool(name="wdram", bufs=1, space="DRAM"))

        # constants / inputs resident in SBUF
        idxh_sb = constp.tile([128, F], I32, tag="idxh")
        idxt_sb = constp.tile([128, F], I32, tag="idxt")
        nc.sync.dma_start(out=idxh_sb[:], in_=idxh[:])
        nc.sync.dma_start(out=idxt_sb[:], in_=idxt[:])
        w1ht_sb = constp.tile([D, H], F32, tag="w1ht")
        w1tt_sb = constp.tile([D, H], F32, tag="w1tt")
        rbt_sb = constp.tile([N_REL, 2 * H], BF16, tag="rbt")
        w2c_sb = constp.tile([96, 2], F32, tag="w2c")
        b2b_sb = constp.tile([128, 1], F32, tag="b2b")
        ident = constp.tile([128, 128], F32, tag="ident")
        from concourse.masks import make_identity
        make_identity(nc, ident[:])
        nc.sync.dma_start(out=w1ht_sb[:], in_=w1ht[:])
        nc.sync.dma_start(out=w1tt_sb[:], in_=w1tt[:])
        nc.sync.dma_start(out=rbt_sb[:], in_=rbt[:])
        nc.sync.dma_start(out=w2c_sb[:], in_=w2c[:])
        nc.sync.dma_start(out=b2b_sb[:], in_=b2b[:])

        w_dram = dramp.tile([EP], F32)

        def _emit_w2(p):
            hsb_p, wp_p, s_p, g_p = p
            nc.tensor.matmul(out=wp_p[32 * s_p:32 * s_p + 1, :],
                             lhsT=w2c_sb[:, 0:1], rhs=hsb_p[:, :512],
                             start=True, stop=False, tile_position=(0, 32 * s_p))
            nc.tensor.matmul(out=wp_p[32 * s_p:32 * s_p + 1, :],
                             lhsT=w2c_sb[:, 1:2], rhs=hsb_p[:, 512:],
                             start=False, stop=True, tile_position=(0, 32 * s_p))
            if s_p == 3:
                w_sb = wsbp.tile([128, 512], F32, tag="wsb")
                nc.vector.tensor_copy(out=w_sb[:], in_=wp_p[:])
                nc.sync.dma_start(
                    out=w_dram[g_p * GROUP:(g_p + 1) * GROUP].rearrange("(a b) -> a b", a=4),
                    in_=w_sb[0:128:32, :])

        pending = None
        for g in range(NG):
            gh = gathp.tile([128, KIDX * D], F32, tag="gh")
            gt = gathp.tile([128, KIDX * D], F32, tag="gt")
            # HW indirect DMA consumes ONE index per output partition, so each
            # call gathers 128 rows (one 256B row per partition).
            for j in range(KIDX):
                nc.gpsimd.indirect_dma_start(
                    out=gh[:, j * D:(j + 1) * D], out_offset=None, in_=tab[:],
                    in_offset=bass.IndirectOffsetOnAxis(
                        ap=idxh_sb[:, g * KIDX + j:g * KIDX + j + 1], axis=0))
                nc.gpsimd.indirect_dma_start(
                    out=gt[:, j * D:(j + 1) * D], out_offset=None, in_=tab[:],
                    in_offset=bass.IndirectOffsetOnAxis(
                        ap=idxt_sb[:, g * KIDX + j:g * KIDX + j + 1], axis=0))
            oh = onep.tile([N_REL, GROUP], BF16, tag="oh")
            nc.sync.dma_start(out=oh[:], in_=onehot[g])

            wp = wps.tile([128, 512], F32, tag="wp")
            nc.vector.memset(wp[:], 0.0)
            for s in range(4):
                pend = pending
                # PE transposes: 8 x [128,128] fp32, two PSUM banks
                xpsh = xpp.tile([128, 512], F32, tag="xps")
                xpst = xpp.tile([128, 512], F32, tag="xps")
                for b in range(4):
                    blk = 4 * s + b
                    nc.tensor.transpose(
                        out=xpsh[:, b * 128:(b + 1) * 128],
                        in_=gh[:, blk * D:(blk + 1) * D],
                        identity=ident[:])
                    nc.tensor.transpose(
                        out=xpst[:, b * 128:(b + 1) * 128],
                        in_=gt[:, blk * D:(blk + 1) * D],
                        identity=ident[:])
                xsb = xtp.tile([128, 1024], F32, tag="xsb")
                nc.vector.tensor_copy(out=xsb[:, :512], in_=xpsh[:])
                nc.scalar.activation(out=xsb[:, 512:], in_=xpst[:],
                                     func=mybir.ActivationFunctionType.Copy)
                xh = xsb[:, :512]
                xt_ = xsb[:, 512:]
                hsb = hsbp.tile([96, 1024], F32, tag="hsb")
                for half in range(2):
                    c0 = half * 96
                    hp = hps.tile([96, 512], F32, tag=f"h{half}")
                    nc.tensor.matmul(out=hp[:], lhsT=w1ht_sb[:, c0:c0 + 96],
                                     rhs=xh[:], start=True, stop=False)
                    nc.tensor.matmul(out=hp[:], lhsT=w1tt_sb[:, c0:c0 + 96],
                                     rhs=xt_[:], start=False, stop=False)
                    nc.tensor.matmul(out=hp[:], lhsT=rbt_sb[:, c0:c0 + 96],
                                     rhs=oh[:, s * 512:(s + 1) * 512],
                                     start=False, stop=False)
                    nc.tensor.matmul(out=hp[:], lhsT=rbt_sb[:, H + c0:H + c0 + 96],
                                     rhs=oh[:, s * 512:(s + 1) * 512],
                                     start=False, stop=True)
                    nc.scalar.activation(out=hsb[:, half * 512:(half + 1) * 512],
                                         in_=hp[:], func=RELU)
                # W2 for the PREVIOUS chunk (software pipeline: its relu
                # has completed during this chunk's transposes/W1 matmuls, so
                # the in-order PE stream doesn't stall on ACT).
                if pend is not None:
                    _emit_w2(pend)
                pending = (hsb, wp, s, g)
        if pending is not None:
            _emit_w2(pending)
            pending = None

        tc.strict_bb_all_engine_barrier()

        # final gating: gate = sigmoid(2*(ln(eps) - ln(1-eps) + w + b2))
        wst = finp.tile([128, F], F32, tag="wst")
        ut = finp.tile([128, F], F32, tag="ut")
        l1 = finp.tile([128, F], F32, tag="l1")
        l2 = finp.tile([128, F], F32, tag="l2")
        gt_ = finp.tile([128, F], F32, tag="gt")
        lnb1 = finp.tile([128, 1], F32, tag="lnb1")
        lnb2 = finp.tile([128, 1], F32, tag="lnb2")
        nc.vector.memset(lnb1[:], float(1.0 - BIAS))
        nc.vector.memset(lnb2[:], float(BIAS))
        nc.sync.dma_start(out=wst[:], in_=w_dram[:].rearrange("(p f) -> p f", p=128))
        nc.sync.dma_start(out=ut[:], in_=u_in[:].rearrange("(p f) -> p f", p=128))
        nc.scalar.activation(out=l1[:], in_=ut[:], func=LN,
                             scale=float(2.0 * BIAS - 1.0), bias=lnb1[:])
        nc.scalar.activation(out=l2[:], in_=ut[:], func=LN,
                             scale=float(1.0 - 2.0 * BIAS), bias=lnb2[:])
        nc.vector.tensor_tensor(out=l1[:], in0=l1[:], in1=l2[:],
                                op=mybir.AluOpType.subtract)
        nc.vector.tensor_tensor(out=l1[:], in0=l1[:], in1=wst[:],
                                op=mybir.AluOpType.add)
        nc.scalar.activation(out=gt_[:], in_=l1[:], func=SIG,
                             scale=float(1.0 / TEMP), bias=b2b_sb[:])
        nc.sync.dma_start(out=gate[:].rearrange("(p f) -> p f", p=128), in_=gt_[:])

    nc.compile()
    return nc


def _pos_to_e():
    """Device output position -> padded edge index, per core."""
    pos = np.arange(EP)
    g, r = pos // GROUP, pos % GROUP
    s, r2 = r // 512, r % 512
    b, p = r2 // 128, r2 % 128
    return g * GROUP + 16 * p + 4 * s + b  # e(g,s,b,p) with j = 4s+b


def _prep(edge_index, edge_type, all_embed, relation_emb, u, W1, b1, W2, b2):
    tab32 = np.ascontiguousarray(np.asarray(all_embed, np.float32))
    W1 = np.asarray(W1, np.float32)
    w1ht = np.ascontiguousarray(W1[:, :D].T)
    w1tt = np.ascontiguousarray(W1[:, D:2 * D].T)
    rb = np.asarray(relation_emb, np.float32) @ W1[:, 2 * D:].T + np.asarray(b1, np.float32)
    import ml_dtypes
    rb_hi = rb.astype(ml_dtypes.bfloat16)
    rb_lo = (rb - rb_hi.astype(np.float32)).astype(ml_dtypes.bfloat16)
    rbt = np.ascontiguousarray(np.concatenate([rb_hi, rb_lo], axis=1))  # [32, 384] bf16
    W2 = np.asarray(W2, np.float32)
    w2c = np.ascontiguousarray(np.stack([W2[0, :96], W2[0, 96:]], axis=1).astype(np.float32))
    b2b = np.full((128, 1), 2.0 * float(np.asarray(b2).reshape(-1)[0]), np.float32)

    head = np.asarray(edge_index[0], np.int64).astype(np.int32)
    tail = np.asarray(edge_index[1], np.int64).astype(np.int32)
    etype = np.asarray(edge_type, np.int64).astype(np.int32)
    u = np.asarray(u, np.float32)
    e2p = _pos_to_e()

    in_maps = []
    for c in range(NCORES):
        sl = slice(c * EC, (c + 1) * EC)
        hp = np.zeros(EP, np.int32); hp[:EC] = head[sl]
        tp = np.zeros(EP, np.int32); tp[:EC] = tail[sl]
        ep_ = np.zeros(EP, np.int32); ep_[:EC] = etype[sl]
        up = np.full(EP, 0.5, np.float32); up[:EC] = u[sl]
        idxh = np.ascontiguousarray(hp.reshape(NG, 128, KIDX).transpose(1, 0, 2).reshape(128, F))
        idxt = np.ascontiguousarray(tp.reshape(NG, 128, KIDX).transpose(1, 0, 2).reshape(128, F))
        t_pos = ep_[e2p]
        onehot = (t_pos.reshape(NG, 1, GROUP) ==
                  np.arange(N_REL, dtype=np.int32).reshape(1, N_REL, 1)).astype(__import__('ml_dtypes').bfloat16)
        u_dev = up[e2p]
        in_maps.append({
            "tab": tab32, "idxh": idxh, "idxt": idxt,
            "onehot": onehot, "u": u_dev,
            "w1ht": w1ht, "w1tt": w1tt, "rbt": rbt, "w2c": w2c, "b2b": b2b,
        })
    return in_maps, e2p


def kernel(edge_index, edge_type, all_embed, relation_emb, u, W1, b1, W2, b2):
    if "nc" not in _CACHE:
        _CACHE["nc"] = _build_program()
    nc = _CACHE["nc"]
    in_maps, e2p = _prep(edge_index, edge_type, all_embed, relation_emb, u,
                         W1, b1, W2, b2)
    res = run_bass_kernel_spmd(nc, in_maps, list(range(NCORES)))
    out = np.empty(E, np.float32)
    for c in range(NCORES):
        gate_pos = res.results[c]["gate"]          # [EP] in pos order
        core = np.empty(EP, np.float32)
        core[e2p] = gate_pos
        out[c * EC:(c + 1) * EC] = core[:EC]
    return out



# revision 5
# speedup vs baseline: 4.0260x; 4.0260x over previous
"""Trainium2 Bass kernel for nn_DropLearner (gnn_message_passing).

aug_edge_weight = sigmoid((logit(eps) + MLP([head|tail|rel])) / T)

Strategy (8 NeuronCores, data parallel over edges):
  - Edges globally sorted by (head_bucket, tail_bucket) with 4 node
    buckets of 25000 rows, dealt round-robin to cores so every core has
    the same 16 segment capacities (one compiled program for all cores).
  - Node table stored fp16; head/tail rows fetched with SWDGE dma_gather
    (InstDMAGatherAnt) in transpose mode: each instruction gathers up to
    896 rows (descriptor-ring limit) straight into feature-major
    [128, n] SBUF tiles -- no PE transposes, int16 local indices per
    bucket (hence the bucket sort).
  - MLP fp16: per 512-edge chunk, h.T[2x96, 512] accumulates in PSUM
    from W1h.T @ headT + W1t.T @ tailT + rb.T @ onehot(type); relu'd to
    fp16 SBUF by ACT (half A) and DVE (half B); weight = W2 @ h via
    matmuls packed 4 chunks per PSUM tile (tile_position row packing).
  - Weights staged to DRAM, re-read [128, S/128] for bulk gating
    (Ln/sigmoid on ACT).
fp16 end to end (fp32 PSUM accumulation) -> ~5e-3 max relative error.
"""
import sys
sys.path.insert(0, "/opt/trn_rl_repo")

import contextlib
import numpy as np

import concourse.bacc as bacc
import concourse.bass as bass
import concourse.mybir as mybir
import concourse.tile as tile
from concourse.bass_utils import run_bass_kernel_spmd
from concourse.library_config import mlp as mlp_lib

# ---- problem constants (hardcoded per task contract) ----
N_NODES = 100000
D = 128           # node dim
N_REL = 32
E = 500000
H = 192           # 3 * mlp_dim
TEMP = 0.5
BIAS = 1e-4

NCORES = 8
NB = 4            # node buckets (int16 gather indices => <= 32768 rows)
BUCKET = N_NODES // NB
NIDX_MAX = 896    # SWDGE ring limit for 256B-row transpose gathers
GTILE = 12288     # slots per SBUF gather tile (24 chunks of 512)
CHUNK = 512

F16 = mybir.dt.float16
F32 = mybir.dt.float32
I16 = mybir.dt.int16

_CACHE = {}


def _make_plan(caps):
    """Static gather-instruction plan from the 16 segment capacities.

    Returns (S, instrs) where instrs is a list of
    (side, bucket, slot_start, n); n is a multiple of 128 and <= NIDX_MAX,
    and no instruction crosses a GTILE boundary.
    """
    offs = np.concatenate([[0], np.cumsum(caps)]).astype(int)
    S = int(offs[-1])
    assert S % CHUNK == 0
    head_runs = [(b, int(offs[4 * b]), int(offs[4 * (b + 1)])) for b in range(NB)]
    tail_runs = [(s % NB, int(offs[s]), int(offs[s + 1])) for s in range(16)]
    instrs = []
    for side, runs in (("h", head_runs), ("t", tail_runs)):
        for b, a, e in runs:
            pos = a
            while pos < e:
                gend = (pos // GTILE + 1) * GTILE
                n = min(NIDX_MAX, min(e, gend) - pos)
                instrs.append((side, b, pos, n))
                pos += n
    return S, instrs


def _build_program(caps):
    S, instrs = _make_plan(caps)
    F = S // 128
    NGRP = (S + GTILE - 1) // GTILE
    TC = sum(n // 16 for _, _, _, n in instrs)  # total idx columns

    nc = bacc.Bacc("TRN2", target_bir_lowering=False, debug=False,
                   num_devices=NCORES)
    tab = nc.dram_tensor("tab", [N_NODES, D], F16, kind="ExternalInput").ap()
    idxs = nc.dram_tensor("idxs", [128, TC], I16, kind="ExternalInput").ap()
    onehot = nc.dram_tensor("onehot", [N_REL, S], F16, kind="ExternalInput").ap()
    u_in = nc.dram_tensor("u", [S], F32, kind="ExternalInput").ap()
    w1ht = nc.dram_tensor("w1ht", [D, H], F16, kind="ExternalInput").ap()
    w1tt = nc.dram_tensor("w1tt", [D, H], F16, kind="ExternalInput").ap()
    rbt = nc.dram_tensor("rbt", [N_REL, H], F16, kind="ExternalInput").ap()
    w2c = nc.dram_tensor("w2c", [96, 2], F16, kind="ExternalInput").ap()
    b2b = nc.dram_tensor("b2b", [128, 1], F32, kind="ExternalInput").ap()
    gate = nc.dram_tensor("gate", [S], F32, kind="ExternalOutput").ap()

    RELU = mybir.ActivationFunctionType.Relu
    LN = mybir.ActivationFunctionType.Ln
    SIG = mybir.ActivationFunctionType.Sigmoid

    # instructions grouped by the GTILE group they write
    per_group = [[] for _ in range(NGRP)]
    col = 0
    for side, b, pos, n in instrs:
        per_group[pos // GTILE].append((side, b, pos, n, col))
        col += n // 16

    with tile.TileContext(nc) as tc, contextlib.ExitStack() as ctx:
        constp = ctx.enter_context(tc.tile_pool(name="const", bufs=1))
        gathp = ctx.enter_context(tc.tile_pool(name="gath", bufs=2))
        onep = ctx.enter_context(tc.tile_pool(name="onep", bufs=2))
        hps = ctx.enter_context(tc.tile_pool(name="hps", bufs=2, space="PSUM"))
        wps = ctx.enter_context(tc.tile_pool(name="wps", bufs=2, space="PSUM"))
        hsbp = ctx.enter_context(tc.tile_pool(name="hsb", bufs=3))
        wsbp = ctx.enter_context(tc.tile_pool(name="wsb", bufs=2))
        finp = ctx.enter_context(tc.tile_pool(name="fin", bufs=1))
        dramp = ctx.enter_context(tc.tile_pool(name="wdram", bufs=1, space="DRAM"))

        nc.gpsimd.load_library(mlp_lib)

        idx_sb = constp.tile([128, TC], I16, tag="idx")
        nc.sync.dma_start(out=idx_sb[:], in_=idxs[:])
        w1ht_sb = constp.tile([D, H], F16, tag="w1ht")
        w1tt_sb = constp.tile([D, H], F16, tag="w1tt")
        rbt_sb = constp.tile([N_REL, H], F16, tag="rbt")
        w2c_sb = constp.tile([96, 2], F16, tag="w2c")
        b2b_sb = constp.tile([128, 1], F32, tag="b2b")
        nc.sync.dma_start(out=w1ht_sb[:], in_=w1ht[:])
        nc.sync.dma_start(out=w1tt_sb[:], in_=w1tt[:])
        nc.sync.dma_start(out=rbt_sb[:], in_=rbt[:])
        nc.sync.dma_start(out=w2c_sb[:], in_=w2c[:])
        nc.sync.dma_start(out=b2b_sb[:], in_=b2b[:])

        w_dram = dramp.tile([S], F32)

        def _emit_w2(p):
            hsb_p, wp_p, s_p, base_p, nch_p = p
            nc.tensor.matmul(out=wp_p[32 * s_p:32 * s_p + 1, :],
                             lhsT=w2c_sb[:, 0:1], rhs=hsb_p[:, :CHUNK],
                             start=True, stop=False, tile_position=(0, 32 * s_p))
            nc.tensor.matmul(out=wp_p[32 * s_p:32 * s_p + 1, :],
                             lhsT=w2c_sb[:, 1:2], rhs=hsb_p[:, CHUNK:],
                             start=False, stop=True, tile_position=(0, 32 * s_p))
            if s_p == nch_p - 1:
                w_sb = wsbp.tile([128, CHUNK], F32, tag="wsb")
                nc.vector.tensor_copy(out=w_sb[:], in_=wp_p[:])
                nc.sync.dma_start(
                    out=w_dram[base_p:base_p + nch_p * CHUNK]
                        .rearrange("(a b) -> a b", a=nch_p),
                    in_=w_sb[0:32 * nch_p:32, :])

        pending = None
        for g in range(NGRP):
            g0 = g * GTILE
            gsz = min(GTILE, S - g0)
            gh = gathp.tile([128, GTILE], F16, tag="gh")
            gt = gathp.tile([128, GTILE], F16, tag="gt")
            for side, b, pos, n, icol in per_group[g]:
                dst = (gh if side == "h" else gt)[:, pos - g0:pos - g0 + n]
                nc.gpsimd.dma_gather(
                    out_ap=dst.rearrange("p (o n) -> p o n", o=1),
                    in_ap=tab[b * BUCKET:(b + 1) * BUCKET, :],
                    idxs_ap=idx_sb[:, icol:icol + n // 16],
                    num_idxs=n,
                    num_idxs_reg=n,
                    elem_size=D,
                    transpose=True,
                )
            oh = onep.tile([N_REL, GTILE], F16, tag="oh")
            nc.sync.dma_start(out=oh[:, :gsz], in_=onehot[:, g0:g0 + gsz])

            nchunks = gsz // CHUNK
            for s in range(nchunks):
                c0 = s * CHUNK
                spk = s % 4                    # position in the 4-chunk pack
                if spk == 0:
                    wp = wps.tile([128, CHUNK], F32, tag="wp")
                    base = g0 + c0
                    npack = min(4, nchunks - s)
                xh = gh[:, c0:c0 + CHUNK]
                xt = gt[:, c0:c0 + CHUNK]
                xo = oh[:, c0:c0 + CHUNK]
                hsb = hsbp.tile([96, 2 * CHUNK], F16, tag="hsb")
                for half in range(2):
                    h0 = half * 96
                    hp = hps.tile([96, CHUNK], F32, tag=f"h{half}")
                    nc.tensor.matmul(out=hp[:], lhsT=w1ht_sb[:, h0:h0 + 96],
                                     rhs=xh, start=True, stop=False)
                    nc.tensor.matmul(out=hp[:], lhsT=w1tt_sb[:, h0:h0 + 96],
                                     rhs=xt, start=False, stop=False)
                    nc.tensor.matmul(out=hp[:], lhsT=rbt_sb[:, h0:h0 + 96],
                                     rhs=xo, start=False, stop=True)
                    if half == 0:
                        nc.scalar.activation(
                            out=hsb[:, :CHUNK], in_=hp[:], func=RELU)
                    else:
                        nc.vector.tensor_scalar_max(
                            out=hsb[:, CHUNK:], in0=hp[:], scalar1=0.0)
                # W2 for the PREVIOUS chunk (software pipeline: its relu has
                # completed during this chunk's matmuls, so the in-order PE
                # stream doesn't stall on ACT/DVE).
                if pending is not None:
                    _emit_w2(pending)
                pending = (hsb, wp, spk, base, npack)
        if pending is not None:
            _emit_w2(pending)
            pending = None

        tc.strict_bb_all_engine_barrier()

        # final gating: gate = sigmoid(2*(ln(eps) - ln(1-eps) + w + b2))
        wst = finp.tile([128, F], F32, tag="wst")
        ut = finp.tile([128, F], F32, tag="ut")
        l1 = finp.tile([128, F], F32, tag="l1")
        l2 = finp.tile([128, F], F32, tag="l2")
        gt_ = finp.tile([128, F], F32, tag="gt")
        lnb1 = finp.tile([128, 1], F32, tag="lnb1")
        lnb2 = finp.tile([128, 1], F32, tag="lnb2")
        nc.vector.memset(lnb1[:], float(1.0 - BIAS))
        nc.vector.memset(lnb2[:], float(BIAS))
        nc.sync.dma_start(out=wst[:], in_=w_dram[:].rearrange("(p f) -> p f", p=128))
        nc.sync.dma_start(out=ut[:], in_=u_in[:].rearrange("(p f) -> p f", p=128))
        nc.scalar.activation(out=l1[:], in_=ut[:], func=LN,
                             scale=float(2.0 * BIAS - 1.0), bias=lnb1[:])
        nc.scalar.activation(out=l2[:], in_=ut[:], func=LN,
                             scale=float(1.0 - 2.0 * BIAS), bias=lnb2[:])
        nc.vector.tensor_tensor(out=l1[:], in0=l1[:], in1=l2[:],
                                op=mybir.AluOpType.subtract)
        nc.vector.tensor_tensor(out=l1[:], in0=l1[:], in1=wst[:],
                                op=mybir.AluOpType.add)
        nc.scalar.activation(out=gt_[:], in_=l1[:], func=SIG,
                             scale=float(1.0 / TEMP), bias=b2b_sb[:])
        nc.sync.dma_start(out=gate[:].rearrange("(p f) -> p f", p=128), in_=gt_[:])

    nc.compile()
    return nc, S, instrs


def _prep(edge_index, edge_type, all_embed, relation_emb, u, W1, b1, W2, b2):
    tab16 = np.ascontiguousarray(np.asarray(all_embed, np.float32).astype(np.float16))
    W1 = np.asarray(W1, np.float32)
    w1ht = np.ascontiguousarray(W1[:, :D].T.astype(np.float16))
    w1tt = np.ascontiguousarray(W1[:, D:2 * D].T.astype(np.float16))
    rb = np.asarray(relation_emb, np.float32) @ W1[:, 2 * D:].T + np.asarray(b1, np.float32)
    rbt = np.ascontiguousarray(rb.astype(np.float16))          # [32, 192]
    W2 = np.asarray(W2, np.float32)
    w2c = np.ascontiguousarray(
        np.stack([W2[0, :96], W2[0, 96:]], axis=1).astype(np.float16))
    b2b = np.full((128, 1), 2.0 * float(np.asarray(b2).reshape(-1)[0]), np.float32)

    head = np.asarray(edge_index[0]).astype(np.int64)
    tail = np.asarray(edge_index[1]).astype(np.int64)
    etype = np.asarray(edge_type).astype(np.int64)
    u = np.asarray(u, np.float32)

    # global (head_bucket, tail_bucket) sort; deal each segment round-robin
    seg = (head // BUCKET) * NB + tail // BUCKET
    order = np.argsort(seg, kind="stable")
    seg_sorted = seg[order]
    seg_counts = np.bincount(seg_sorted, minlength=16)
    seg_starts = np.concatenate([[0], np.cumsum(seg_counts)])[:16]

    per_core_n = [int(-(-c // NCORES)) for c in seg_counts]    # ceil
    caps = [(-(-n // 128)) * 128 for n in per_core_n]
    # pad last nonempty cap so S is a CHUNK multiple
    S0 = sum(caps)
    caps[15] += (-S0) % CHUNK
    S, instrs = _make_plan(caps)
    offs = np.concatenate([[0], np.cumsum(caps)]).astype(int)

    # per-core slot arrays
    hloc = np.zeros((NCORES, S), np.int16)
    tloc = np.zeros((NCORES, S), np.int16)
    typ = np.zeros((NCORES, S), np.int64)
    u_dev = np.full((NCORES, S), 0.5, np.float32)
    eid = np.full((NCORES, S), -1, np.int64)
    for s16 in range(16):
        hb, tb = s16 // NB, s16 % NB
        eg = order[seg_starts[s16]:seg_starts[s16] + seg_counts[s16]]
        for c in range(NCORES):
            mine = eg[c::NCORES]
            n = len(mine)
            sl = slice(offs[s16], offs[s16] + n)
            hloc[c, sl] = (head[mine] - hb * BUCKET).astype(np.int16)
            tloc[c, sl] = (tail[mine] - tb * BUCKET).astype(np.int16)
            typ[c, sl] = etype[mine]
            u_dev[c, sl] = u[mine]
            eid[c, sl] = mine
            # padding rows use local index 0 of the segment's buckets (valid)

    # idx buffer: per instruction, local indices wrapped [16, n/16],
    # replicated across the 8 Q7-core partition stripes
    TC = sum(n // 16 for _, _, _, n in instrs)
    idxv = np.zeros((NCORES, 128, TC), np.int16)
    col = 0
    for side, b, pos, n in instrs:
        loc = hloc if side == "h" else tloc
        blk = loc[:, pos:pos + n].reshape(NCORES, n // 16, 16).transpose(0, 2, 1)
        idxv[:, :, col:col + n // 16] = np.tile(blk, (1, 8, 1))
        col += n // 16

    onehot = (typ[:, None, :] == np.arange(N_REL, dtype=np.int64)[None, :, None]
              ).astype(np.float16)                              # [NCORES, 32, S]

    in_maps = []
    for c in range(NCORES):
        in_maps.append({
            "tab": tab16, "idxs": np.ascontiguousarray(idxv[c]),
            "onehot": np.ascontiguousarray(onehot[c]), "u": u_dev[c],
            "w1ht": w1ht, "w1tt": w1tt, "rbt": rbt, "w2c": w2c, "b2b": b2b,
        })
    return caps, in_maps, eid


def kernel(edge_index, edge_type, all_embed, relation_emb, u, W1, b1, W2, b2):
    caps, in_maps, eid = _prep(edge_index, edge_type, all_embed, relation_emb,
                               u, W1, b1, W2, b2)
    key = tuple(caps)
    if key not in _CACHE:
        _CACHE.clear()
        _CACHE[key] = _build_program(caps)
    nc, S, _ = _CACHE[key]
    res = run_bass_kernel_spmd(nc, in_maps, list(range(NCORES)))
    out = np.empty(E, np.float32)
    for c in range(NCORES):
        g = np.asarray(res.results[c]["gate"])
        valid = eid[c] >= 0
        out[eid[c][valid]] = g[valid]
    return out


# revision 15
# speedup vs baseline: 4.3666x; 1.0846x over previous
"""Trainium2 Bass kernel for nn_DropLearner (gnn_message_passing).

aug_edge_weight = sigmoid((logit(eps) + MLP([head|tail|rel])) / T)

Strategy (8 NeuronCores, data parallel over edges):
  - Edges globally sorted by (head_bucket, tail_bucket) with 4 node
    buckets of 25000 rows, dealt round-robin to cores so every core has
    the same 16 segment capacities (one compiled program for all cores).
  - Node table stored fp16; head/tail rows fetched with SWDGE dma_gather
    (InstDMAGatherAnt) in transpose mode: each instruction gathers up to
    896 rows (descriptor-ring limit) straight into feature-major
    [128, n] SBUF tiles -- no PE transposes, int16 local indices per
    bucket (hence the bucket sort).
  - MLP fp16: per 512-edge chunk, h.T[2x96, 512] accumulates in PSUM
    from W1h.T @ headT + W1t.T @ tailT + rb.T @ onehot(type); relu'd to
    fp16 SBUF by ACT (half A) and DVE (half B); weight = W2 @ h via
    matmuls packed 4 chunks per PSUM tile (tile_position row packing).
  - Weights staged to DRAM, re-read [128, S/128] for bulk gating
    (Ln/sigmoid on ACT).
fp16 end to end (fp32 PSUM accumulation) -> ~5e-3 max relative error.
"""
import sys
sys.path.insert(0, "/opt/trn_rl_repo")

import contextlib
import numpy as np

import concourse.bacc as bacc
import concourse.bass as bass
import concourse.mybir as mybir
import concourse.tile as tile
from concourse.bass_utils import run_bass_kernel_spmd
from concourse.library_config import mlp as mlp_lib

# ---- problem constants (hardcoded per task contract) ----
N_NODES = 100000
D = 128           # node dim
N_REL = 32
E = 500000
H = 192           # 3 * mlp_dim
TEMP = 0.5
BIAS = 1e-4

NCORES = 8
NB = 4            # node buckets (int16 gather indices => <= 32768 rows)
BUCKET = N_NODES // NB
NIDX_MAX = 896    # SWDGE ring limit for 256B-row transpose gathers
GTILE = 12288     # max slots per SBUF gather tile (24 chunks of 512)
CHUNK = 512


def _group_edges(S):
    """Group boundaries: small groups at the start (pipeline fill) and end
    (drain), GTILE in the middle. All sizes are CHUNK multiples."""
    ramp_up = [2048, 4096, 8192]
    ramp_dn = [4096, 2048]
    sizes = []
    rest = S
    for g in ramp_up:
        if rest <= g:
            break
        sizes.append(g)
        rest -= g
    tail = []
    for g in ramp_dn:
        if rest > g:
            tail.append(g)
            rest -= g
    nmid = max(0, -(-rest // GTILE))
    mid = []
    if nmid:
        base = rest // nmid // CHUNK * CHUNK
        mid = [base] * nmid
        mid[0] += rest - base * nmid
    sizes += mid + tail
    assert sum(sizes) == S and all(x % CHUNK == 0 for x in sizes)
    edges = np.concatenate([[0], np.cumsum(sizes)]).astype(int)
    return edges

F16 = mybir.dt.float16
F32 = mybir.dt.float32
I16 = mybir.dt.int16

_CACHE = {}


def _make_plan(caps):
    """Static gather-instruction plan from the 16 segment capacities.

    Returns (S, instrs) where instrs is a list of
    (side, bucket, slot_start, n); n is a multiple of 128 and <= NIDX_MAX,
    and no instruction crosses a GTILE boundary.
    """
    offs = np.concatenate([[0], np.cumsum(caps)]).astype(int)
    S = int(offs[-1])
    assert S % CHUNK == 0
    edges = _group_edges(S)
    head_runs = [(b, int(offs[4 * b]), int(offs[4 * (b + 1)])) for b in range(NB)]
    tail_runs = [(s % NB, int(offs[s]), int(offs[s + 1])) for s in range(16)]
    instrs = []
    for side, runs in (("h", head_runs), ("t", tail_runs)):
        for b, a, e in runs:
            pos = a
            while pos < e:
                gend = int(edges[np.searchsorted(edges, pos, side="right")])
                n = min(NIDX_MAX, min(e, gend) - pos)
                instrs.append((side, b, pos, n))
                pos += n
    return S, edges, instrs


def _build_program(caps):
    S, edges, instrs = _make_plan(caps)
    F = S // 128
    NGRP = len(edges) - 1
    TC = sum(n // 16 for _, _, _, n in instrs)  # total idx columns

    nc = bacc.Bacc("TRN2", target_bir_lowering=False, debug=False,
                   num_devices=NCORES)
    tab = nc.dram_tensor("tab", [N_NODES, D], F16, kind="ExternalInput").ap()
    idxs = nc.dram_tensor("idxs", [128, TC], I16, kind="ExternalInput").ap()
    onehot = nc.dram_tensor("onehot", [N_REL, S], F16, kind="ExternalInput").ap()
    u_in = nc.dram_tensor("u", [S], F32, kind="ExternalInput").ap()
    w1ht = nc.dram_tensor("w1ht", [D, H], F16, kind="ExternalInput").ap()
    w1tt = nc.dram_tensor("w1tt", [D, H], F16, kind="ExternalInput").ap()
    rbt = nc.dram_tensor("rbt", [N_REL, H], F16, kind="ExternalInput").ap()
    w2c = nc.dram_tensor("w2c", [96, 2], F16, kind="ExternalInput").ap()
    b2b = nc.dram_tensor("b2b", [128, 1], F32, kind="ExternalInput").ap()
    gate = nc.dram_tensor("gate", [S], F32, kind="ExternalOutput").ap()

    RELU = mybir.ActivationFunctionType.Relu
    LN = mybir.ActivationFunctionType.Ln
    SIG = mybir.ActivationFunctionType.Sigmoid

    # instructions grouped by the group tile they write, interleaved by
    # position so early chunks of each group become ready first
    per_group = [[] for _ in range(NGRP)]
    col = 0
    for side, b, pos, n in instrs:
        g = int(np.searchsorted(edges, pos, side="right")) - 1
        per_group[g].append((side, b, pos, n, col))
        col += n // 16
    for lst in per_group:
        lst.sort(key=lambda t: (t[2] + t[3], t[0]))

    with tile.TileContext(nc) as tc, contextlib.ExitStack() as ctx:
        constp = ctx.enter_context(tc.tile_pool(name="const", bufs=1))
        gathp = ctx.enter_context(tc.tile_pool(name="gath", bufs=2))
        onep = ctx.enter_context(tc.tile_pool(name="onep", bufs=2))
        hps = ctx.enter_context(tc.tile_pool(name="hps", bufs=2, space="PSUM"))
        wps = ctx.enter_context(tc.tile_pool(name="wps", bufs=2, space="PSUM"))
        hsbp = ctx.enter_context(tc.tile_pool(name="hsb", bufs=3))
        wsbp = ctx.enter_context(tc.tile_pool(name="wsb", bufs=2))
        finp = ctx.enter_context(tc.tile_pool(name="fin", bufs=1))
        dramp = ctx.enter_context(tc.tile_pool(name="wdram", bufs=1, space="DRAM"))

        nc.gpsimd.load_library(mlp_lib)

        idx_sb = constp.tile([128, TC], I16, tag="idx")
        for c0 in range(0, TC, 1024):
            c1 = min(TC, c0 + 1024)
            nc.sync.dma_start(out=idx_sb[:, c0:c1], in_=idxs[:, c0:c1])
        w1ht_sb = constp.tile([D, H], F16, tag="w1ht")
        w1tt_sb = constp.tile([D, H], F16, tag="w1tt")
        rbt_sb = constp.tile([N_REL, H], F16, tag="rbt")
        w2c_sb = constp.tile([96, 2], F16, tag="w2c")
        b2b_sb = constp.tile([128, 1], F32, tag="b2b")
        nc.sync.dma_start(out=w1ht_sb[:], in_=w1ht[:])
        nc.sync.dma_start(out=w1tt_sb[:], in_=w1tt[:])
        nc.sync.dma_start(out=rbt_sb[:], in_=rbt[:])
        nc.sync.dma_start(out=w2c_sb[:], in_=w2c[:])
        nc.sync.dma_start(out=b2b_sb[:], in_=b2b[:])

        w_dram = dramp.tile([S], F32)

        # logit(eps) precompute off the critical path (Ln table first, before
        # the relu loop claims the ACT table): logit = ln(eps) - ln(1-eps)
        logit = finp.tile([128, F], F32, tag="logit")
        ut = finp.tile([128, F], F32, tag="ut")
        l2 = finp.tile([128, F], F32, tag="l2")
        lnb1 = finp.tile([128, 1], F32, tag="lnb1")
        lnb2 = finp.tile([128, 1], F32, tag="lnb2")
        nc.vector.memset(lnb1[:], float(1.0 - BIAS))
        nc.vector.memset(lnb2[:], float(BIAS))
        nc.sync.dma_start(out=ut[:], in_=u_in[:].rearrange("(p f) -> p f", p=128))
        nc.scalar.activation(out=logit[:], in_=ut[:], func=mybir.ActivationFunctionType.Ln,
                             scale=float(2.0 * BIAS - 1.0), bias=lnb1[:])
        nc.scalar.activation(out=l2[:], in_=ut[:], func=mybir.ActivationFunctionType.Ln,
                             scale=float(1.0 - 2.0 * BIAS), bias=lnb2[:])
        nc.vector.tensor_tensor(out=logit[:], in0=logit[:], in1=l2[:],
                                op=mybir.AluOpType.subtract)

        def _emit_w2(p):
            hsb_p, wp_p, s_p, base_p, nch_p = p
            nc.tensor.matmul(out=wp_p[32 * s_p:32 * s_p + 1, :],
                             lhsT=w2c_sb[:, 0:1], rhs=hsb_p[:, :CHUNK],
                             start=True, stop=False, tile_position=(0, 32 * s_p))
            nc.tensor.matmul(out=wp_p[32 * s_p:32 * s_p + 1, :],
                             lhsT=w2c_sb[:, 1:2], rhs=hsb_p[:, CHUNK:],
                             start=False, stop=True, tile_position=(0, 32 * s_p))
            if s_p == nch_p - 1:
                w_sb = wsbp.tile([128, CHUNK], F32, tag="wsb")
                nc.vector.tensor_copy(out=w_sb[:], in_=wp_p[:])
                nc.sync.dma_start(
                    out=w_dram[base_p:base_p + nch_p * CHUNK]
                        .rearrange("(a b) -> a b", a=nch_p),
                    in_=w_sb[0:32 * nch_p:32, :])

        pending = None
        for g in range(NGRP):
            g0 = int(edges[g])
            gsz = int(edges[g + 1]) - g0
            gh = gathp.tile([128, GTILE], F16, tag="gh")
            gt = gathp.tile([128, GTILE], F16, tag="gt")
            for side, b, pos, n, icol in per_group[g]:
                dst = (gh if side == "h" else gt)[:, pos - g0:pos - g0 + n]
                nc.gpsimd.dma_gather(
                    out_ap=dst.rearrange("p (o n) -> p o n", o=1),
                    in_ap=tab[b * BUCKET:(b + 1) * BUCKET, :],
                    idxs_ap=idx_sb[:, icol:icol + n // 16],
                    num_idxs=n,
                    num_idxs_reg=n,
                    elem_size=D,
                    transpose=True,
                )
            oh = onep.tile([N_REL, GTILE], F16, tag="oh")
            nc.sync.dma_start(out=oh[:, :gsz], in_=onehot[:, g0:g0 + gsz])

            nchunks = gsz // CHUNK
            for s in range(nchunks):
                c0 = s * CHUNK
                spk = s % 4                    # position in the 4-chunk pack
                if spk == 0:
                    wp = wps.tile([128, CHUNK], F32, tag="wp")
                    base = g0 + c0
                    npack = min(4, nchunks - s)
                xh = gh[:, c0:c0 + CHUNK]
                xt = gt[:, c0:c0 + CHUNK]
                xo = oh[:, c0:c0 + CHUNK]
                hsb = hsbp.tile([96, 2 * CHUNK], F16, tag="hsb")
                for half in range(2):
                    h0 = half * 96
                    hp = hps.tile([96, CHUNK], F32, tag=f"h{half}")
                    nc.tensor.matmul(out=hp[:], lhsT=w1ht_sb[:, h0:h0 + 96],
                                     rhs=xh, start=True, stop=False)
                    nc.tensor.matmul(out=hp[:], lhsT=w1tt_sb[:, h0:h0 + 96],
                                     rhs=xt, start=False, stop=False)
                    nc.tensor.matmul(out=hp[:], lhsT=rbt_sb[:, h0:h0 + 96],
                                     rhs=xo, start=False, stop=True)
                    if half == 0:
                        nc.scalar.activation(
                            out=hsb[:, :CHUNK], in_=hp[:], func=RELU)
                    else:
                        nc.vector.tensor_scalar_max(
                            out=hsb[:, CHUNK:], in0=hp[:], scalar1=0.0)
                # W2 for the PREVIOUS chunk (software pipeline: its relu has
                # completed during this chunk's matmuls, so the in-order PE
                # stream doesn't stall on ACT/DVE).
                if pending is not None:
                    _emit_w2(pending)
                pending = (hsb, wp, spk, base, npack)
        if pending is not None:
            _emit_w2(pending)
            pending = None

        # final gating: gate = sigmoid(2*(logit + w + b2)), pipelined in
        # partition-row bands (band b covers the contiguous slot interval
        # [64*b*F, 64*(b+1)*F) of w_dram, so the DMA read depends only on the
        # w writes for those slots -- no global barrier needed)
        wst = finp.tile([128, F], F32, tag="wst")
        gt_ = finp.tile([128, F], F32, tag="gt")
        w2d = w_dram[:].rearrange("(p f) -> p f", p=128)
        g2d = gate[:].rearrange("(p f) -> p f", p=128)
        for b in range(2):
            r = slice(64 * b, 64 * (b + 1))
            nc.sync.dma_start(out=wst[r, :], in_=w2d[r, :])
            nc.vector.tensor_tensor(out=wst[r, :], in0=wst[r, :],
                                    in1=logit[r, :], op=mybir.AluOpType.add)
            nc.scalar.activation(out=gt_[r, :], in_=wst[r, :], func=SIG,
                                 scale=float(1.0 / TEMP), bias=b2b_sb[r, :])
            nc.sync.dma_start(out=g2d[r, :], in_=gt_[r, :])

    nc.compile()
    return nc, S, instrs


def _prep(edge_index, edge_type, all_embed, relation_emb, u, W1, b1, W2, b2):
    tab16 = np.ascontiguousarray(np.asarray(all_embed, np.float32).astype(np.float16))
    W1 = np.asarray(W1, np.float32)
    w1ht = np.ascontiguousarray(W1[:, :D].T.astype(np.float16))
    w1tt = np.ascontiguousarray(W1[:, D:2 * D].T.astype(np.float16))
    rb = np.asarray(relation_emb, np.float32) @ W1[:, 2 * D:].T + np.asarray(b1, np.float32)
    rbt = np.ascontiguousarray(rb.astype(np.float16))          # [32, 192]
    W2 = np.asarray(W2, np.float32)
    w2c = np.ascontiguousarray(
        np.stack([W2[0, :96], W2[0, 96:]], axis=1).astype(np.float16))
    b2b = np.full((128, 1), 2.0 * float(np.asarray(b2).reshape(-1)[0]), np.float32)

    head = np.asarray(edge_index[0]).astype(np.int64)
    tail = np.asarray(edge_index[1]).astype(np.int64)
    etype = np.asarray(edge_type).astype(np.int64)
    u = np.asarray(u, np.float32)

    # global (head_bucket, tail_bucket) sort; deal each segment round-robin
    seg = (head // BUCKET) * NB + tail // BUCKET
    order = np.argsort(seg, kind="stable")
    seg_sorted = seg[order]
    seg_counts = np.bincount(seg_sorted, minlength=16)
    seg_starts = np.concatenate([[0], np.cumsum(seg_counts)])[:16]

    per_core_n = [int(-(-c // NCORES)) for c in seg_counts]    # ceil
    caps = [(-(-n // 128)) * 128 for n in per_core_n]
    # pad last nonempty cap so S is a CHUNK multiple
    S0 = sum(caps)
    caps[15] += (-S0) % CHUNK
    S, _, instrs = _make_plan(caps)
    offs = np.concatenate([[0], np.cumsum(caps)]).astype(int)

    # per-core slot arrays
    hloc = np.zeros((NCORES, S), np.int16)
    tloc = np.zeros((NCORES, S), np.int16)
    typ = np.zeros((NCORES, S), np.int64)
    u_dev = np.full((NCORES, S), 0.5, np.float32)
    eid = np.full((NCORES, S), -1, np.int64)
    for s16 in range(16):
        hb, tb = s16 // NB, s16 % NB
        eg = order[seg_starts[s16]:seg_starts[s16] + seg_counts[s16]]
        for c in range(NCORES):
            mine = eg[c::NCORES]
            n = len(mine)
            sl = slice(offs[s16], offs[s16] + n)
            hloc[c, sl] = (head[mine] - hb * BUCKET).astype(np.int16)
            tloc[c, sl] = (tail[mine] - tb * BUCKET).astype(np.int16)
            typ[c, sl] = etype[mine]
            u_dev[c, sl] = u[mine]
            eid[c, sl] = mine
            # padding rows use local index 0 of the segment's buckets (valid)

    # idx buffer: per instruction, local indices wrapped [16, n/16],
    # replicated across the 8 Q7-core partition stripes
    TC = sum(n // 16 for _, _, _, n in instrs)
    idxv = np.zeros((NCORES, 128, TC), np.int16)
    col = 0
    for side, b, pos, n in instrs:
        loc = hloc if side == "h" else tloc
        blk = loc[:, pos:pos + n].reshape(NCORES, n // 16, 16).transpose(0, 2, 1)
        idxv[:, :, col:col + n // 16] = np.tile(blk, (1, 8, 1))
        col += n // 16

    onehot = (typ[:, None, :] == np.arange(N_REL, dtype=np.int64)[None, :, None]
              ).astype(np.float16)                              # [NCORES, 32, S]

    in_maps = []
    for c in range(NCORES):
        in_maps.append({
            "tab": tab16, "idxs": np.ascontiguousarray(idxv[c]),
            "onehot": np.ascontiguousarray(onehot[c]), "u": u_dev[c],
            "w1ht": w1ht, "w1tt": w1tt, "rbt": rbt, "w2c": w2c, "b2b": b2b,
        })
    return caps, in_maps, eid


def kernel(edge_index, edge_type, all_embed, relation_emb, u, W1, b1, W2, b2):
    caps, in_maps, eid = _prep(edge_index, edge_type, all_embed, relation_emb,
                               u, W1, b1, W2, b2)
    key = tuple(caps)
    if key not in _CACHE:
        _CACHE.clear()
        _CACHE[key] = _build_program(caps)
    nc, S, _ = _CACHE[key]
    res = run_bass_kernel_spmd(nc, in_maps, list(range(NCORES)))
    out = np.empty(E, np.float32)
    for c in range(NCORES):
        g = np.asarray(res.results[c]["gate"])
        valid = eid[c] >= 0
        out[eid[c][valid]] = g[valid]
    return out


# revision 23
# speedup vs baseline: 4.4208x; 1.0124x over previous
"""Trainium2 Bass kernel for nn_DropLearner (gnn_message_passing).

aug_edge_weight = sigmoid((logit(eps) + MLP([head|tail|rel])) / T)

Strategy (8 NeuronCores, data parallel over edges):
  - Edges globally sorted by (head_bucket, tail_bucket) with 4 node
    buckets of 25000 rows, dealt round-robin to cores so every core has
    the same 16 segment capacities (one compiled program for all cores).
  - Node table stored fp16; head/tail rows fetched with SWDGE dma_gather
    (InstDMAGatherAnt) in transpose mode: each instruction gathers up to
    896 rows (descriptor-ring limit) straight into feature-major
    [128, n] SBUF tiles -- no PE transposes, int16 local indices per
    bucket (hence the bucket sort).
  - MLP fp16: per 512-edge chunk, h.T[2x96, 512] accumulates in PSUM
    from W1h.T @ headT + W1t.T @ tailT + rb.T @ onehot(type); relu'd to
    fp16 SBUF by ACT (half A) and DVE (half B); weight = W2 @ h via
    matmuls packed 4 chunks per PSUM tile (tile_position row packing).
  - Weights staged to DRAM, re-read [128, S/128] for bulk gating
    (Ln/sigmoid on ACT).
fp16 end to end (fp32 PSUM accumulation) -> ~5e-3 max relative error.
"""
import sys
sys.path.insert(0, "/opt/trn_rl_repo")

import contextlib
import numpy as np

import concourse.bacc as bacc
import concourse.bass as bass
import concourse.mybir as mybir
import concourse.tile as tile
from concourse.bass_utils import run_bass_kernel_spmd
from concourse.library_config import mlp as mlp_lib

# ---- problem constants (hardcoded per task contract) ----
N_NODES = 100000
D = 128           # node dim
N_REL = 32
E = 500000
H = 192           # 3 * mlp_dim
TEMP = 0.5
BIAS = 1e-4

NCORES = 8
NB = 4            # node buckets (int16 gather indices => <= 32768 rows)
BUCKET = N_NODES // NB
NIDX_MAX = 896    # SWDGE ring limit for 256B-row transpose gathers
GTILE = 12288     # max slots per SBUF gather tile (24 chunks of 512)
CHUNK = 512


def _group_edges(S):
    """Group boundaries: small groups at the start (pipeline fill) and end
    (drain), GTILE in the middle. All sizes are CHUNK multiples."""
    ramp_up = [2048, 4096, 8192]
    ramp_dn = [4096, 2048]
    sizes = []
    rest = S
    for g in ramp_up:
        if rest <= g:
            break
        sizes.append(g)
        rest -= g
    tail = []
    for g in ramp_dn:
        if rest > g:
            tail.append(g)
            rest -= g
    nmid = max(0, -(-rest // GTILE))
    mid = []
    if nmid:
        base = rest // nmid // CHUNK * CHUNK
        mid = [base] * nmid
        mid[0] += rest - base * nmid
    sizes += mid + tail
    assert sum(sizes) == S and all(x % CHUNK == 0 for x in sizes)
    edges = np.concatenate([[0], np.cumsum(sizes)]).astype(int)
    return edges

F16 = mybir.dt.float16
F32 = mybir.dt.float32
I16 = mybir.dt.int16

_CACHE = {}


def _make_plan(caps):
    """Static gather-instruction plan from the 16 segment capacities.

    Returns (S, instrs) where instrs is a list of
    (side, bucket, slot_start, n); n is a multiple of 128 and <= NIDX_MAX,
    and no instruction crosses a GTILE boundary.
    """
    offs = np.concatenate([[0], np.cumsum(caps)]).astype(int)
    S = int(offs[-1])
    assert S % CHUNK == 0
    edges = _group_edges(S)
    head_runs = [(b, int(offs[4 * b]), int(offs[4 * (b + 1)])) for b in range(NB)]
    tail_runs = [(s % NB, int(offs[s]), int(offs[s + 1])) for s in range(16)]
    instrs = []
    for side, runs in (("h", head_runs), ("t", tail_runs)):
        for b, a, e in runs:
            pos = a
            while pos < e:
                gend = int(edges[np.searchsorted(edges, pos, side="right")])
                n = min(NIDX_MAX, min(e, gend) - pos)
                g = int(np.searchsorted(edges, pos, side="right")) - 1
                instrs.append((side, b, pos, n, g))
                pos += n
    # final emission order: by group, interleaved by end position so early
    # chunks of each group become ready first; idx columns follow this order
    instrs.sort(key=lambda t: (t[4], t[2] + t[3], t[0]))
    return S, edges, instrs


def _build_program(caps):
    S, edges, instrs = _make_plan(caps)
    F = S // 128
    NGRP = len(edges) - 1
    TC = sum(n // 16 for _, _, _, n, _ in instrs)  # total idx columns

    nc = bacc.Bacc("TRN2", target_bir_lowering=False, debug=False,
                   num_devices=NCORES)
    tab = nc.dram_tensor("tab", [N_NODES, D], F16, kind="ExternalInput").ap()
    idxs = nc.dram_tensor("idxs", [128, TC], I16, kind="ExternalInput").ap()
    onehot = nc.dram_tensor("onehot", [N_REL, S], F16, kind="ExternalInput").ap()
    u_in = nc.dram_tensor("u", [S], F32, kind="ExternalInput").ap()
    w1ht = nc.dram_tensor("w1ht", [D, H], F16, kind="ExternalInput").ap()
    w1tt = nc.dram_tensor("w1tt", [D, H], F16, kind="ExternalInput").ap()
    rbt = nc.dram_tensor("rbt", [N_REL, H], F16, kind="ExternalInput").ap()
    w2c = nc.dram_tensor("w2c", [96, 2], F16, kind="ExternalInput").ap()
    b2b = nc.dram_tensor("b2b", [128, 1], F32, kind="ExternalInput").ap()
    gate = nc.dram_tensor("gate", [S], F32, kind="ExternalOutput").ap()

    RELU = mybir.ActivationFunctionType.Relu
    LN = mybir.ActivationFunctionType.Ln
    SIG = mybir.ActivationFunctionType.Sigmoid

    # instrs is already in emission order (grouped, interleaved); assign
    # idx columns sequentially and split per group
    per_group = [[] for _ in range(NGRP)]
    gcol = [[TC, 0] for _ in range(NGRP)]      # per-group idx col range
    col = 0
    for side, b, pos, n, g in instrs:
        per_group[g].append((side, b, pos, n, col))
        gcol[g][0] = min(gcol[g][0], col)
        col += n // 16
        gcol[g][1] = max(gcol[g][1], col)

    with tile.TileContext(nc) as tc, contextlib.ExitStack() as ctx:
        constp = ctx.enter_context(tc.tile_pool(name="const", bufs=1))
        gathp = ctx.enter_context(tc.tile_pool(name="gath", bufs=2))
        onep = ctx.enter_context(tc.tile_pool(name="onep", bufs=2))
        hps = ctx.enter_context(tc.tile_pool(name="hps", bufs=2, space="PSUM"))
        wps = ctx.enter_context(tc.tile_pool(name="wps", bufs=2, space="PSUM"))
        hsbp = ctx.enter_context(tc.tile_pool(name="hsb", bufs=3))
        wsbp = ctx.enter_context(tc.tile_pool(name="wsb", bufs=2))
        finp = ctx.enter_context(tc.tile_pool(name="fin", bufs=1))
        dramp = ctx.enter_context(tc.tile_pool(name="wdram", bufs=1, space="DRAM"))

        nc.gpsimd.load_library(mlp_lib)

        # idx columns load pipelined: groups 0-2 up front, group g+3 after
        # group g's gathers (the big single DMA otherwise delays the first
        # gather transfers on the shared DMA engines)
        idx_sb = constp.tile([128, TC], I16, tag="idx")

        def _load_idx(g_lo, g_hi):
            g_hi = min(g_hi, NGRP)
            if g_lo >= g_hi:
                return
            c0 = min(gcol[g][0] for g in range(g_lo, g_hi))
            c1 = max(gcol[g][1] for g in range(g_lo, g_hi))
            if c1 > c0:
                nc.sync.dma_start(out=idx_sb[:, c0:c1], in_=idxs[:, c0:c1])

        _load_idx(0, 3)
        w1ht_sb = constp.tile([D, H], F16, tag="w1ht")
        w1tt_sb = constp.tile([D, H], F16, tag="w1tt")
        rbt_sb = constp.tile([N_REL, H], F16, tag="rbt")
        w2c_sb = constp.tile([96, 2], F16, tag="w2c")
        b2b_sb = constp.tile([128, 1], F32, tag="b2b")
        nc.sync.dma_start(out=w1ht_sb[:], in_=w1ht[:])
        nc.sync.dma_start(out=w1tt_sb[:], in_=w1tt[:])
        nc.sync.dma_start(out=rbt_sb[:], in_=rbt[:])
        nc.sync.dma_start(out=w2c_sb[:], in_=w2c[:])
        nc.sync.dma_start(out=b2b_sb[:], in_=b2b[:])

        w_dram = dramp.tile([S], F32)

        # logit(eps) precompute off the critical path (Ln table first, before
        # the relu loop claims the ACT table): logit = ln(eps) - ln(1-eps)
        logit = finp.tile([128, F], F32, tag="logit")
        ut = finp.tile([128, F], F32, tag="ut")
        l2 = finp.tile([128, F], F32, tag="l2")
        lnb1 = finp.tile([128, 1], F32, tag="lnb1")
        lnb2 = finp.tile([128, 1], F32, tag="lnb2")
        nc.vector.memset(lnb1[:], float(1.0 - BIAS))
        nc.vector.memset(lnb2[:], float(BIAS))
        nc.sync.dma_start(out=ut[:], in_=u_in[:].rearrange("(p f) -> p f", p=128))
        nc.scalar.activation(out=logit[:], in_=ut[:], func=mybir.ActivationFunctionType.Ln,
                             scale=float(2.0 * BIAS - 1.0), bias=lnb1[:])
        nc.scalar.activation(out=l2[:], in_=ut[:], func=mybir.ActivationFunctionType.Ln,
                             scale=float(1.0 - 2.0 * BIAS), bias=lnb2[:])
        nc.vector.tensor_tensor(out=logit[:], in0=logit[:], in1=l2[:],
                                op=mybir.AluOpType.subtract)

        def _emit_w2(p):
            hsb_p, wp_p, s_p, base_p, nch_p = p
            nc.tensor.matmul(out=wp_p[32 * s_p:32 * s_p + 1, :],
                             lhsT=w2c_sb[:, 0:1], rhs=hsb_p[:, :CHUNK],
                             start=True, stop=False, tile_position=(0, 32 * s_p))
            nc.tensor.matmul(out=wp_p[32 * s_p:32 * s_p + 1, :],
                             lhsT=w2c_sb[:, 1:2], rhs=hsb_p[:, CHUNK:],
                             start=False, stop=True, tile_position=(0, 32 * s_p))
            if s_p == nch_p - 1:
                w_sb = wsbp.tile([128, CHUNK], F32, tag="wsb")
                nc.vector.tensor_copy(out=w_sb[:], in_=wp_p[:])
                nc.sync.dma_start(
                    out=w_dram[base_p:base_p + nch_p * CHUNK]
                        .rearrange("(a b) -> a b", a=nch_p),
                    in_=w_sb[0:32 * nch_p:32, :])

        pending = None
        for g in range(NGRP):
            g0 = int(edges[g])
            gsz = int(edges[g + 1]) - g0
            gh = gathp.tile([128, GTILE], F16, tag="gh")
            gt = gathp.tile([128, GTILE], F16, tag="gt")
            for side, b, pos, n, icol in per_group[g]:
                dst = (gh if side == "h" else gt)[:, pos - g0:pos - g0 + n]
                nc.gpsimd.dma_gather(
                    out_ap=dst.rearrange("p (o n) -> p o n", o=1),
                    in_ap=tab[b * BUCKET:(b + 1) * BUCKET, :],
                    idxs_ap=idx_sb[:, icol:icol + n // 16],
                    num_idxs=n,
                    num_idxs_reg=n,
                    elem_size=D,
                    transpose=True,
                )
            _load_idx(g + 3, g + 4)
            oh = onep.tile([N_REL, GTILE], F16, tag="oh")
            nc.sync.dma_start(out=oh[:, :gsz], in_=onehot[:, g0:g0 + gsz])

            nchunks = gsz // CHUNK
            for s in range(nchunks):
                c0 = s * CHUNK
                spk = s % 4                    # position in the 4-chunk pack
                if spk == 0:
                    wp = wps.tile([128, CHUNK], F32, tag="wp")
                    base = g0 + c0
                    npack = min(4, nchunks - s)
                xh = gh[:, c0:c0 + CHUNK]
                xt = gt[:, c0:c0 + CHUNK]
                xo = oh[:, c0:c0 + CHUNK]
                hsb = hsbp.tile([96, 2 * CHUNK], F16, tag="hsb")
                for half in range(2):
                    h0 = half * 96
                    hp = hps.tile([96, CHUNK], F32, tag=f"h{half}")
                    nc.tensor.matmul(out=hp[:], lhsT=w1ht_sb[:, h0:h0 + 96],
                                     rhs=xh, start=True, stop=False)
                    nc.tensor.matmul(out=hp[:], lhsT=w1tt_sb[:, h0:h0 + 96],
                                     rhs=xt, start=False, stop=False)
                    nc.tensor.matmul(out=hp[:], lhsT=rbt_sb[:, h0:h0 + 96],
                                     rhs=xo, start=False, stop=True)
                    if half == 0:
                        nc.scalar.activation(
                            out=hsb[:, :CHUNK], in_=hp[:], func=RELU)
                    else:
                        nc.vector.tensor_scalar_max(
                            out=hsb[:, CHUNK:], in0=hp[:], scalar1=0.0)
                # W2 for the PREVIOUS chunk (software pipeline: its relu has
                # completed during this chunk's matmuls, so the in-order PE
                # stream doesn't stall on ACT/DVE).
                if pending is not None:
                    _emit_w2(pending)
                pending = (hsb, wp, spk, base, npack)
        if pending is not None:
            _emit_w2(pending)
            pending = None

        # final gating: gate = sigmoid(2*(logit + w + b2)), pipelined in
        # partition-row bands (band b covers the contiguous slot interval
        # [64*b*F, 64*(b+1)*F) of w_dram, so the DMA read depends only on the
        # w writes for those slots -- no global barrier needed)
        wst = finp.tile([128, F], F32, tag="wst")
        gt_ = finp.tile([128, F], F32, tag="gt")
        w2d = w_dram[:].rearrange("(p f) -> p f", p=128)
        g2d = gate[:].rearrange("(p f) -> p f", p=128)
        for b in range(2):
            r = slice(64 * b, 64 * (b + 1))
            nc.sync.dma_start(out=wst[r, :], in_=w2d[r, :])
            nc.vector.tensor_tensor(out=wst[r, :], in0=wst[r, :],
                                    in1=logit[r, :], op=mybir.AluOpType.add)
            nc.scalar.activation(out=gt_[r, :], in_=wst[r, :], func=SIG,
                                 scale=float(1.0 / TEMP), bias=b2b_sb[r, :])
            nc.sync.dma_start(out=g2d[r, :], in_=gt_[r, :])

    nc.compile()
    return nc, S, instrs


def _prep(edge_index, edge_type, all_embed, relation_emb, u, W1, b1, W2, b2):
    tab16 = np.ascontiguousarray(np.asarray(all_embed, np.float32).astype(np.float16))
    W1 = np.asarray(W1, np.float32)
    w1ht = np.ascontiguousarray(W1[:, :D].T.astype(np.float16))
    w1tt = np.ascontiguousarray(W1[:, D:2 * D].T.astype(np.float16))
    rb = np.asarray(relation_emb, np.float32) @ W1[:, 2 * D:].T + np.asarray(b1, np.float32)
    rbt = np.ascontiguousarray(rb.astype(np.float16))          # [32, 192]
    W2 = np.asarray(W2, np.float32)
    w2c = np.ascontiguousarray(
        np.stack([W2[0, :96], W2[0, 96:]], axis=1).astype(np.float16))
    b2b = np.full((128, 1), 2.0 * float(np.asarray(b2).reshape(-1)[0]), np.float32)

    head = np.asarray(edge_index[0]).astype(np.int64)
    tail = np.asarray(edge_index[1]).astype(np.int64)
    etype = np.asarray(edge_type).astype(np.int64)
    u = np.asarray(u, np.float32)

    # global (head_bucket, tail_bucket) sort; deal each segment round-robin
    seg = (head // BUCKET) * NB + tail // BUCKET
    order = np.argsort(seg, kind="stable")
    seg_sorted = seg[order]
    seg_counts = np.bincount(seg_sorted, minlength=16)
    seg_starts = np.concatenate([[0], np.cumsum(seg_counts)])[:16]

    per_core_n = [int(-(-c // NCORES)) for c in seg_counts]    # ceil
    caps = [(-(-n // 128)) * 128 for n in per_core_n]
    # pad last nonempty cap so S is a CHUNK multiple
    S0 = sum(caps)
    caps[15] += (-S0) % CHUNK
    S, _, instrs = _make_plan(caps)
    offs = np.concatenate([[0], np.cumsum(caps)]).astype(int)

    # per-core slot arrays
    hloc = np.zeros((NCORES, S), np.int16)
    tloc = np.zeros((NCORES, S), np.int16)
    typ = np.zeros((NCORES, S), np.int64)
    u_dev = np.full((NCORES, S), 0.5, np.float32)
    eid = np.full((NCORES, S), -1, np.int64)
    for s16 in range(16):
        hb, tb = s16 // NB, s16 % NB
        eg = order[seg_starts[s16]:seg_starts[s16] + seg_counts[s16]]
        for c in range(NCORES):
            mine = eg[c::NCORES]
            n = len(mine)
            sl = slice(offs[s16], offs[s16] + n)
            hloc[c, sl] = (head[mine] - hb * BUCKET).astype(np.int16)
            tloc[c, sl] = (tail[mine] - tb * BUCKET).astype(np.int16)
            typ[c, sl] = etype[mine]
            u_dev[c, sl] = u[mine]
            eid[c, sl] = mine
            # padding rows use local index 0 of the segment's buckets (valid)

    # idx buffer: per instruction, local indices wrapped [16, n/16],
    # replicated across the 8 Q7-core partition stripes
    TC = sum(n // 16 for _, _, _, n, _ in instrs)
    idxv = np.zeros((NCORES, 128, TC), np.int16)
    col = 0
    for side, b, pos, n, _g in instrs:
        loc = hloc if side == "h" else tloc
        blk = loc[:, pos:pos + n].reshape(NCORES, n // 16, 16).transpose(0, 2, 1)
        idxv[:, :, col:col + n // 16] = np.tile(blk, (1, 8, 1))
        col += n // 16

    onehot = (typ[:, None, :] == np.arange(N_REL, dtype=np.int64)[None, :, None]
              ).astype(np.float16)                              # [NCORES, 32, S]

    in_maps = []
    for c in range(NCORES):
        in_maps.append({
            "tab": tab16, "idxs": np.ascontiguousarray(idxv[c]),
            "onehot": np.ascontiguousarray(onehot[c]), "u": u_dev[c],
            "w1ht": w1ht, "w1tt": w1tt, "rbt": rbt, "w2c": w2c, "b2b": b2b,
        })
    return caps, in_maps, eid


def kernel(edge_index, edge_type, all_embed, relation_emb, u, W1, b1, W2, b2):
    caps, in_maps, eid = _prep(edge_index, edge_type, all_embed, relation_emb,
                               u, W1, b1, W2, b2)
    key = tuple(caps)
    if key not in _CACHE:
        _CACHE.clear()
        _CACHE[key] = _build_program(caps)
    nc, S, _ = _CACHE[key]
    res = run_bass_kernel_spmd(nc, in_maps, list(range(NCORES)))
    out = np.empty(E, np.float32)
    for c in range(NCORES):
        g = np.asarray(res.results[c]["gate"])
        valid = eid[c] >= 0
        out[eid[c][valid]] = g[valid]
    return out
